# revision 9
# baseline (speedup 1.0000x reference)
"""CoSTCo model kernel for 8x Trainium2 NeuronCores.

Math: out[b] = relu(wfc2 @ relu(wfc1 @ h2[b] + bfc1) + bfc2), where
  h2[b] = relu(Q02[i0[b]*64 + i2[b]] + Q1[i1[b]])
  Q_m   = relu(emb_m @ w1.T + b1) @ w2[:, :, m].T        (weight folding)
  Q02[i*64+j] = Q0[i] + Q2[j] + b2                       (pair fusion)

conv1 (over rank) and conv2 (over modes) act linearly on each gathered
embedding row, so they fold into per-table lookup matrices computed once
on the host. Modes 0 and 2 fuse into one 21696-row pair table, so the
device does 2 dma_gathers per batch element.

v3: the batch is sorted by (i02, i1) on the host so the pair-table HBM
reads are near-sequential (the inverse permutation is applied on the
host afterwards). Per 512-element block: DVE add, 8 PE transposes into
[channel, batch] PSUM, ACT relu drains, fc1 as 4 N=512 matmuls with DVE
bias+relu drains, and fc2 as 2 matmuls whose [128, 32] weights are zero
except column s, accumulating all 32 block outputs into one [32, 512]
PSUM bank drained once at the end. Index loads are split head/tail so
the first gather starts immediately; fc1/fc2 are emitted one/two blocks
behind the transposes so the PE never waits on ACT/DVE drains.

Sharding: pure data parallel over the batch dim, 16384 elements per core.
"""

import sys
import types

sys.path.insert(0, "/opt/trn_rl_repo")

import ml_dtypes
import numpy as np

# ---------------------------------------------------------------- constants
B = 131072
N_CORES = 8
BPC = B // N_CORES          # 16384 batch elements per core
CHUNK = 1024                # idx per dma_gather instruction (ring cap)
NCHUNK = BPC // CHUNK       # 16
NBLK = BPC // 512           # 32 N=512 matmul blocks
BLK_PER_CHUNK = CHUNK // 512
HEAD_CHUNKS = 1             # chunks whose indices load in the first DMA
RANK = 128
C = 256                     # channels
FIELD_DIMS = (339, 5825, 64)
F02 = FIELD_DIMS[0] * FIELD_DIMS[2]   # fused pair-table rows
NSWQ = 4                    # SWDGE queues in use


def _install_ntff_hook():
    """antenv in this image lacks axon_hooks; inject it and register the
    ctypes NTFF profiling hook so trace=True works under axon."""
    import antenv

    if "antenv.axon_hooks" in sys.modules:
        return
    mod = types.ModuleType("antenv.axon_hooks")
    mod._hook = None
    mod.set_axon_ntff_profile_hook = lambda h: setattr(mod, "_hook", h)
    mod.get_axon_ntff_profile_hook = lambda: mod._hook
    sys.modules["antenv.axon_hooks"] = mod
    antenv.axon_hooks = mod
    try:
        from trn_agent_boot.trn_boot import _ntff_profile_via_ctypes

        mod._hook = _ntff_profile_via_ctypes("/opt/axon/libaxon_pjrt.so")
    except Exception:
        pass


_NC_CACHE = {}


def _build():
    """Build + compile the per-core Bass program. Identical on all cores;
    per-core data arrives via in_maps."""
    import concourse.bass as bass
    import concourse.tile as tile
    from concourse import bacc, mybir

    key = "v4"
    if key in _NC_CACHE:
        return _NC_CACHE[key]

    f32 = mybir.dt.float32
    bf16 = mybir.dt.bfloat16
    i16 = mybir.dt.int16
    Alu = mybir.AluOpType
    Act = mybir.ActivationFunctionType

    hcols = HEAD_CHUNKS * CHUNK // 16          # idx cols in the head tiles
    tcols = BPC // 16 - hcols
    ngrp = CHUNK // 128                        # gather row groups per chunk

    nc = bacc.Bacc("TRN2", target_bir_lowering=False, debug=False,
                   num_devices=N_CORES, num_swdge_queues=NSWQ,
                   dynamic_dma_scratch_size=65536)

    q02_dram = nc.dram_tensor("q02", [F02, C], bf16, kind="ExternalInput")
    q1_dram = nc.dram_tensor("q1", [FIELD_DIMS[1], C], bf16,
                             kind="ExternalInput")
    idx_dram = nc.dram_tensor("idxw", [2, 128, BPC // 16], i16,
                              kind="ExternalInput")
    WCOLS = 4 * 128 + NBLK * 2 * 32 + 128      # w1 | w2z | ident
    wc_dram = nc.dram_tensor("wc", [128, WCOLS], bf16, kind="ExternalInput")
    bc_dram = nc.dram_tensor("bc", [128, 3], f32, kind="ExternalInput")
    out_dram = nc.dram_tensor("out", [NBLK, 512], f32, kind="ExternalOutput")

    with tile.TileContext(nc) as tc:
        with (
            tc.tile_pool(name="const", bufs=1) as const_pool,
            tc.tile_pool(name="g02", bufs=6) as g02_pool,
            tc.tile_pool(name="g1", bufs=6) as g1_pool,
            tc.tile_pool(name="sum", bufs=4) as sum_pool,
            tc.tile_pool(name="h2", bufs=4) as h2_pool,
            tc.tile_pool(name="h3", bufs=3) as h3_pool,
            tc.tile_pool(name="pt", bufs=4, space="PSUM") as pt_pool,
            tc.tile_pool(name="ph", bufs=3, space="PSUM") as ph_pool,
            tc.tile_pool(name="po", bufs=1, space="PSUM") as po_pool,
        ):
            # --- index head tiles first so chunk-0 gathers start ASAP
            idxh, idxt = [], []
            for m in range(2):
                th = const_pool.tile([128, hcols], i16, tag=f"idxh{m}")
                nc.sync.dma_start(th[:], idx_dram.ap()[m][:, :hcols])
                idxh.append(th)
            wcs = const_pool.tile([128, WCOLS], bf16, tag="wc")
            nc.sync.dma_start(wcs[:], wc_dram.ap())
            bcs = const_pool.tile([128, 3], f32, tag="bc")
            nc.sync.dma_start(bcs[:], bc_dram.ap())
            w1s = wcs[:, 0:4 * 128]
            w2s = wcs[:, 4 * 128:4 * 128 + NBLK * 2 * 32]
            ident = wcs[:, 4 * 128 + NBLK * 2 * 32:]
            b1s = bcs[:, 0:2]
            for m in range(2):
                tt = const_pool.tile([128, tcols], i16, tag=f"idxt{m}")
                nc.sync.dma_start(tt[:], idx_dram.ap()[m][:, hcols:])
                idxt.append(tt)
            po = po_pool.tile([128, 512], f32, tag="po")
            stage = const_pool.tile([32, 512], f32, tag="stage")

            nidx_reg = nc.gpsimd.to_reg(CHUNK)   # one shared reg, one MOVE
            ccols = CHUNK // 16        # idx cols per gather instruction
            g02s = [None] * NCHUNK
            g1s = [None] * NCHUNK
            h2s = [None] * NBLK
            h3s = [None] * NBLK

            def idx_slice(m, ch):
                if ch < HEAD_CHUNKS:
                    return idxh[m][:, ch * ccols:(ch + 1) * ccols]
                c = ch - HEAD_CHUNKS
                return idxt[m][:, c * ccols:(c + 1) * ccols]

            def emit_gathers(ch):
                g02 = g02_pool.tile([128, ngrp, C], bf16, tag="g02")
                nc.gpsimd.dma_gather(
                    g02[:], q02_dram.ap(), idx_slice(0, ch),
                    CHUNK, nidx_reg, C, queue_num=(2 * ch) % NSWQ)
                g02s[ch] = g02
                g1 = g1_pool.tile([128, ngrp, C], bf16, tag="g1")
                nc.gpsimd.dma_gather(
                    g1[:], q1_dram.ap(), idx_slice(1, ch),
                    CHUNK, nidx_reg, C, queue_num=(2 * ch + 1) % NSWQ)
                g1s[ch] = g1

            def emit_add_transpose_relu(s):
                """sum + 8 PE transposes + 2 ACT relu drains for block s."""
                ch, half = divmod(s, BLK_PER_CHUNK)
                gs = slice(4 * half, 4 * half + 4)
                sm = sum_pool.tile([128, 4, C], bf16, tag="sum")
                nc.vector.tensor_tensor(sm[:], g02s[ch][:, gs, :],
                                        g1s[ch][:, gs, :], Alu.add)
                h2 = h2_pool.tile([128, 2, 512], bf16, tag="h2")
                for h in range(2):
                    ps = pt_pool.tile([128, 512], bf16, tag="pt")
                    for grp in range(4):
                        nc.tensor.transpose(
                            ps[:, grp * 128:(grp + 1) * 128],
                            sm[:, grp, h * 128:(h + 1) * 128],
                            ident,
                        )
                    nc.scalar.activation(h2[:, h, :], ps[:], Act.Relu)
                h2s[s] = h2

            def emit_fc1(s):
                h3 = h3_pool.tile([128, 2, 512], bf16, tag="h3")
                for h in range(2):
                    ph = ph_pool.tile([128, 512], f32, tag="ph")
                    for j in range(2):
                        nc.tensor.matmul(
                            ph[:],
                            w1s[:, (j * 2 + h) * 128:(j * 2 + h + 1) * 128],
                            h2s[s][:, j, :],
                            start=(j == 0), stop=(j == 1),
                        )
                    if h == 0:
                        nc.vector.tensor_scalar(h3[:, h, :], ph[:],
                                                b1s[:, h:h + 1], 0.0,
                                                Alu.add, Alu.max)
                    else:
                        nc.scalar.activation(h3[:, h, :], ph[:], Act.Relu,
                                             bias=b1s[:, h:h + 1])
                h3s[s] = h3

            def emit_fc2(s):
                for j in range(2):
                    nc.tensor.matmul(
                        po[0:32, :],
                        w2s[:, (s * 2 + j) * 32:(s * 2 + j + 1) * 32],
                        h3s[s][:, j, :],
                        start=(s == 0 and j == 0),
                        stop=(s == NBLK - 1 and j == 1),
                        skip_group_check=True,
                    )

            # software-pipelined emission: transposes for block s overlap
            # fc1 for s-1 and fc2 for s-2 on the PE queue
            for s in range(NBLK):
                if s % BLK_PER_CHUNK == 0:
                    emit_gathers(s // BLK_PER_CHUNK)
                emit_add_transpose_relu(s)
                if s >= 1:
                    emit_fc1(s - 1)
                if s >= 2:
                    emit_fc2(s - 2)
            emit_fc1(NBLK - 1)
            emit_fc2(NBLK - 2)
            emit_fc2(NBLK - 1)
            # --- final: out = relu(po + b3), one [32, 512] drain + DMA
            nc.scalar.activation(stage[:], po[0:32, :], Act.Relu,
                                 bias=bcs[0:32, 2:3])
            nc.sync.dma_start(out_dram.ap(), stage[:])

    nc.compile()
    _NC_CACHE[key] = nc
    return nc


def _fold_tables(inputs):
    """Q_m = relu(emb_m @ w1.T + b1) @ w2[:,:,m].T in float64, then the
    mode-0/2 pair fusion Q02[i*64+j] = Q0[i] + Q2[j] + b2."""
    w1_ = np.asarray(inputs["w1"]).astype(np.float64)
    b1_ = np.asarray(inputs["b1"]).astype(np.float64)
    w2 = np.asarray(inputs["w2"])
    qs = []
    for m, emb in enumerate((inputs["emb0"], inputs["emb1"], inputs["emb2"])):
        r = np.maximum(np.asarray(emb).astype(np.float64) @ w1_.T + b1_, 0.0)
        qs.append(r @ w2[:, :, m].astype(np.float64).T)
    q02 = (qs[0][:, None, :] + qs[2][None, :, :]
           + np.asarray(inputs["b2"]).astype(np.float64)).reshape(F02, C)
    return q02, qs[1]


def _make_common(inputs):
    bf = ml_dtypes.bfloat16
    q02, q1 = _fold_tables(inputs)
    wfc1 = np.asarray(inputs["wfc1"]).astype(np.float32)   # [256, 256]
    wfc2 = np.asarray(inputs["wfc2"]).astype(np.float32).reshape(C)
    # fc1 lhsT tiles: w1t[:, (j*2+h)*128 + m] = wfc1[128h+m, 128j+k]
    w1t = np.zeros((128, 4 * 128), np.float32)
    for j in range(2):
        for h in range(2):
            blk = wfc1[128 * h:128 * (h + 1), 128 * j:128 * (j + 1)].T
            w1t[:, (j * 2 + h) * 128:(j * 2 + h + 1) * 128] = blk
    # fc2 zero-padded lhsT tiles: column s of tile (s, j) holds wfc2 half j
    w2z = np.zeros((128, NBLK * 2 * 32), np.float32)
    for s in range(NBLK):
        for j in range(2):
            w2z[:, (s * 2 + j) * 32 + s] = wfc2[128 * j:128 * (j + 1)]
    wc = np.concatenate([w1t, w2z, np.eye(128, dtype=np.float32)], axis=1)
    bc = np.zeros((128, 3), np.float32)
    bc[:, 0:2] = np.asarray(inputs["bfc1"]).astype(np.float32).reshape(2, 128).T
    bc[:, 2] = float(np.asarray(inputs["bfc2"])[0])
    return {
        "q02": np.ascontiguousarray(q02.astype(bf)),
        "q1": np.ascontiguousarray(q1.astype(bf)),
        "wc": np.ascontiguousarray(wc.astype(bf)),
        "bc": bc,
    }


def _wrap_idx(idx, chunk):
    """Wrap a 1-D int array into dma_gather's [128, n/16] int16 layout,
    chunk by chunk: logical position k of chunk c lives at
    [k % 16, c*chunk/16 + k // 16], replicated across the 8 Q7 cores."""
    n = idx.shape[0]
    w = (idx.reshape(n // chunk, chunk // 16, 16)
         .transpose(0, 2, 1).reshape(n // chunk, 16, chunk // 16))
    wrapped = np.concatenate(list(w), axis=1).astype(np.int16)  # [16, n/16]
    return np.tile(wrapped, (8, 1))                             # [128, n/16]


def _make_idxw(shard, chunk=CHUNK):
    """shard: [n, 3] int indices -> ([2, 128, n/16] int16 wrapped layout,
    order). Row 0 is the fused mode-0/2 index, row 1 the mode-1 index.
    The batch is sorted by (i02, i1) so table HBM reads are sequential-ish;
    `order` maps device position -> original row (undo with
    out[order] = device_out)."""
    i02 = np.asarray(shard[:, 0]).astype(np.int64) * FIELD_DIMS[2] \
        + np.asarray(shard[:, 2])
    i1 = np.asarray(shard[:, 1]).astype(np.int64)
    order = np.lexsort((i1, i02))
    return np.stack([_wrap_idx(i02[order], chunk),
                     _wrap_idx(i1[order], chunk)]), order


def _run(inputs, trace=False, trace_kwargs=None):
    _install_ntff_hook()
    from concourse.bass_utils import run_bass_kernel_spmd

    nc = _build()
    common = _make_common(inputs)
    indices = np.asarray(inputs["indices"])
    in_maps, orders = [], []
    for c in range(N_CORES):
        shard = indices[c * BPC:(c + 1) * BPC]
        idxw, order = _make_idxw(shard)
        in_maps.append({**common, "idxw": idxw})
        orders.append(order)

    res = run_bass_kernel_spmd(nc, in_maps, core_ids=list(range(N_CORES)),
                               trace=trace, **(trace_kwargs or {}))
    out = np.empty(B, np.float32)
    for c in range(N_CORES):
        out[c * BPC + orders[c]] = res.results[c]["out"].reshape(BPC)
    return out, res


def kernel(**inputs):
    out, _ = _run(inputs, trace=False)
    return out
